# revision 4
# baseline (speedup 1.0000x reference)
"""Trainium2 Bass kernel for nn_CorrelationLayer — PE version.

Math: out[b, 0, i, j] = sum_{c,y,x} f1[b,c,y+i-2,x+j-2] * f2[b,c,y,x]
    = sum_{m',m} VM2[(m',m), d] * G[b][m',m],  G[b] = f1[b]^T f2[b] over C.

Host pre-transposes each core's shard to [C, BL*16] bf16 (HBM traffic
halves; contraction dim C lands on SBUF partitions).

Device pipeline, per core:
  loads: 32 sub-loads (16 packs each), chunk-major; f1 on the sync ring,
    f2 on gpsimd (the Act ring must stay free for evacuation copies —
    each dma_start costs ~0.6us of sequencer issue time and blocks the
    ring FIFO on its semaphore).
  stage 1: batches packed 8 per matmul — one [128c,128]x[128c,128]
    matmul per (c-chunk, pack) computes an 8x8-block Gram whose diagonal
    blocks are the per-batch Grams (off-diagonal garbage is free:
    instruction overhead dominates tiny matmuls).  Chunk 0 copies into
    s1g (Act), chunks 1-3 accumulate (DVE adds).  The evacuation writes
    scatter into layout [16b+m', m*512 + b*64 + g] so the pack index g
    is contiguous.
  gather: 16 strided SBUF->DRAM DMAs (one per (m-half, in-pack batch),
    128B runs, spread over all 3 rings) stage the diagonal blocks in
    [(h,m',ml), b*64+g] layout (SBUF dst APs cannot split the partition
    dim, DRAM is flat), then — after an explicit barrier, since
    cross-queue DMA->DMA ordering on the staging tile proved racy — 2
    contiguous load-backs into S1BIG[h] [(m'*8+ml) part, b*64+g free];
    batch columns come out g-major (host unpermutes).
  stage 2: 2 matmuls: lhsT = constant mask VM2 [128=(m',ml), 16d],
    rhs = S1BIG[h], K=128, N=512 contiguous, accumulating out4[d, j].

Measured: ~80us HW exec (baseline elementwise kernel: 298us); DMA-bound
(16.8 MB/core at the ~300 GB/s effective DMA-engine ceiling = 56us
transfer + ~9us preamble + ~13us endgame tail).
"""

import sys

import ml_dtypes
import numpy as np

sys.path.insert(0, "/opt/trn_rl_repo")

import concourse.bacc as bacc
import concourse.mybir as mybir
import concourse.tile as tile
from concourse import bass_utils

B, C, H, W = 4096, 512, 4, 4
M = H * W                 # 16 spatial positions
NCORES = 8
BL = B // NCORES          # 512 batches per core
NCH = 4                   # c-chunks of 128 partitions
CH = C // NCH             # 128
FW = BL * M               # 8192 free width of an input chunk
SF = FW // 4              # sub-load width (16 packs)
NPK = BL // 8             # 64 packs of 8 batches

_cached_nc = None


def _make_vm2() -> np.ndarray:
    """vm2[h*128 + m'*8 + ml, d]: pattern mask over (m', m=8h+ml) pairs."""
    vm2 = np.zeros((256, 16), np.float32)
    for mp in range(16):
        for m in range(16):
            y, x = m // 4, m % 4
            for i in range(4):
                for j in range(4):
                    yy, xx = y + i - 2, x + j - 2
                    if 0 <= yy < 4 and 0 <= xx < 4 and mp == yy * 4 + xx:
                        h, ml = m // 8, m % 8
                        vm2[h * 128 + mp * 8 + ml, i * 4 + j] = 1.0
    return vm2.astype(ml_dtypes.bfloat16)


def _build():
    nc = bacc.Bacc("TRN2", target_bir_lowering=False, debug=False)
    f1d = nc.dram_tensor("f1t", [C, FW], mybir.dt.bfloat16, kind="ExternalInput").ap()
    f2d = nc.dram_tensor("f2t", [C, FW], mybir.dt.bfloat16, kind="ExternalInput").ap()
    vmd = nc.dram_tensor("vm2", [256, 16], mybir.dt.bfloat16, kind="ExternalInput").ap()
    outd = nc.dram_tensor("out", [16, BL], mybir.dt.float32, kind="ExternalOutput").ap()

    with tile.TileContext(nc) as tc:
        with (
            tc.tile_pool(name="inp", bufs=1) as inp,
            tc.tile_pool(name="s1g", bufs=1) as s1gp,
            tc.tile_pool(name="sb", bufs=1) as sbp,
            tc.tile_pool(name="vmp", bufs=1) as vmp,
            tc.tile_pool(name="stg", bufs=1, space="DRAM") as stgp,
            tc.tile_pool(name="ps", bufs=7, space="PSUM") as psp,
            tc.tile_pool(name="po", bufs=1, space="PSUM") as pop,
        ):
            t1 = [[None] * 4 for _ in range(NCH)]
            t2 = [[None] * 4 for _ in range(NCH)]
            for q in range(NCH):
                for s in range(4):
                    for tlist, name in ((t1, "t1"), (t2, "t2")):
                        tlist[q][s] = inp.tile(
                            [CH, SF], mybir.dt.bfloat16,
                            tag=f"{name}_{q}_{s}", name=f"{name}_{q}_{s}")

            # blocked Gram accumulator, scatter layout:
            # [16b+m' partitions, m*512 + b*64 + g free]
            s1g = s1gp.tile([128, FW], mybir.dt.bfloat16, tag="s1g",
                            name="s1g")
            # gathered diagonal blocks: [(m'*8+ml) part, b*64+g free]
            sbt = [sbp.tile([128, BL], mybir.dt.bfloat16, tag=f"sb{h}",
                            name=f"sb{h}") for h in range(2)]
            # DRAM staging, flat rows (h, m', ml)
            stg = stgp.tile([256, BL], mybir.dt.bfloat16, tag="stg",
                            name="stg")

            def emit_loads(q, s):
                nc.sync.dma_start(
                    out=t1[q][s][:],
                    in_=f1d[q * CH:(q + 1) * CH, s * SF:(s + 1) * SF])
                nc.gpsimd.dma_start(
                    out=t2[q][s][:],
                    in_=f2d[q * CH:(q + 1) * CH, s * SF:(s + 1) * SF])

            vm2t = [vmp.tile([128, 16], mybir.dt.bfloat16, tag=f"vm{h}",
                             name=f"vm{h}") for h in range(2)]

            for q in range(NCH):
                for s in range(4):
                    emit_loads(q, s)
                    if q == 0 and s == 0:
                        for h in range(2):
                            nc.scalar.dma_start(
                                out=vm2t[h][:],
                                in_=vmd[h * 128:(h + 1) * 128, :])
                    for bk in range(4 * s, 4 * s + 4):
                        pt = psp.tile([128, 512], mybir.dt.float32, tag="pt",
                                      name=f"pt_{q}_{bk}")
                        for j in range(4):
                            go = (bk * 4 + j) % 16
                            sl = slice(go * 128, (go + 1) * 128)
                            nc.tensor.matmul(
                                pt[:, j * 128:(j + 1) * 128],
                                t1[q][s][:, sl], t2[q][s][:, sl],
                                start=True, stop=True,
                            )
                        # scatter-evacuate: dst free dims (m, b, pack)
                        src = pt.rearrange("p (j b m) -> p m b j", j=4, b=8,
                                           m=M)
                        dst = s1g.rearrange(
                            "p (m b g) -> p m b g", m=M, b=8,
                            g=NPK)[:, :, :, 4 * bk:4 * bk + 4]
                        if q == 0:
                            nc.scalar.activation(
                                out=dst, in_=src,
                                func=mybir.ActivationFunctionType.Copy)
                        else:
                            nc.vector.tensor_tensor(
                                out=dst, in0=src, in1=dst,
                                op=mybir.AluOpType.add)

            # gather: stg[(h,m',ml), b*64+g] = s1g[16b+m', (8h+ml)*512+b*64+g]
            # (one DMA per (h, b), split over the sync and gpsimd rings)
            sgv = s1g.rearrange("p (hh ml b2 g) -> p hh ml b2 g",
                                hh=2, ml=8, b2=8, g=NPK)
            dv = stg.rearrange("(hh mp ml) (b2 g) -> hh mp ml b2 g",
                               hh=2, mp=M, ml=8, b2=8)
            engs = [nc.sync, nc.gpsimd, nc.scalar]
            ng = 0
            for h in range(2):
                for b in range(8):
                    engs[ng % 3].dma_start(
                        out=dv[h, :, :, b, :],
                        in_=sgv[16 * b:16 * (b + 1), h, :, b, :])
                    ng += 1
            # cross-queue DMA->DMA ordering on the DRAM staging tile is not
            # reliably enforced by the auto-inserted waits; fence explicitly
            tc.strict_bb_all_engine_barrier()
            for h in range(2):
                (nc.sync if h == 0 else nc.gpsimd).dma_start(
                    out=sbt[h][:], in_=stg[128 * h:128 * (h + 1), :])

            # stage 2: 2 accumulating matmuls, K=128, N=512 contiguous
            out4 = pop.tile([16, BL], mybir.dt.float32, tag="out4",
                            name="out4")
            for h in range(2):
                nc.tensor.matmul(
                    out4[:], vm2t[h][:], sbt[h][:],
                    start=(h == 0), stop=(h == 1),
                )

            ot = vmp.tile([16, BL], mybir.dt.float32, tag="ot", name="ot")
            nc.scalar.activation(out=ot[:], in_=out4[:],
                                 func=mybir.ActivationFunctionType.Copy)
            nc.scalar.dma_start(out=outd, in_=ot[:])

    nc.compile()
    return nc


def _get_nc():
    global _cached_nc
    if _cached_nc is None:
        _cached_nc = _build()
    return _cached_nc


def _prep_core(f: np.ndarray, k: int) -> np.ndarray:
    """Core k's shard as [C, BL*16] bf16 (c-major)."""
    s = f[k * BL:(k + 1) * BL]                      # [BL, C, 16]
    return np.ascontiguousarray(
        s.transpose(1, 0, 2).reshape(C, FW)).astype(ml_dtypes.bfloat16)


# batch index of output column j = b*64+g  ->  batch 8*g + b
_JPERM = 8 * (np.arange(BL) % 64) + np.arange(BL) // 64


def kernel(feat1, feat2):
    f1 = np.asarray(feat1, dtype=np.float32).reshape(B, C, M)
    f2 = np.asarray(feat2, dtype=np.float32).reshape(B, C, M)
    vm2 = _make_vm2()
    nc = _get_nc()
    in_maps = [
        {"f1t": _prep_core(f1, k), "f2t": _prep_core(f2, k), "vm2": vm2}
        for k in range(NCORES)
    ]
    res = bass_utils.run_bass_kernel_spmd(nc, in_maps, list(range(NCORES)))
    out = np.empty((B, M), np.float32)
    for k in range(NCORES):
        r = np.asarray(res.results[k]["out"]).T    # [512 j, 16 d]
        out[k * BL + _JPERM] = r
    return np.ascontiguousarray(out).reshape(B, 1, H, W)


# revision 5
# speedup vs baseline: 1.0289x; 1.0289x over previous
"""Trainium2 Bass kernel for nn_CorrelationLayer — PE version.

Math: out[b, 0, i, j] = sum_{c,y,x} f1[b,c,y+i-2,x+j-2] * f2[b,c,y,x]
    = sum_{m',m} VM2[(m',m), d] * G[b][m',m],  G[b] = f1[b]^T f2[b] over C.

Host pre-transposes each core's shard to [C, BL*16] bf16 (HBM traffic
halves; contraction dim C lands on SBUF partitions).

Device pipeline, per core:
  loads: 32 sub-loads (16 packs each), chunk-major; f1 on the sync ring,
    f2 on gpsimd (the Act ring must stay free for evacuation copies —
    each dma_start costs ~0.6us of sequencer issue time and blocks the
    ring FIFO on its semaphore).
  stage 1: batches packed 8 per matmul — one [128c,128]x[128c,128]
    matmul per (c-chunk, pack) computes an 8x8-block Gram whose diagonal
    blocks are the per-batch Grams (off-diagonal garbage is free:
    instruction overhead dominates tiny matmuls).  Chunk 0 copies into
    s1g (Act), chunks 1-3 accumulate (DVE adds).  The evacuation writes
    scatter into layout [16b+m', m*512 + b*64 + g] so the pack index g
    is contiguous.
  gather: 16 strided SBUF->DRAM DMAs (one per (m-half, in-pack batch),
    128B runs, spread over all 3 rings) stage the diagonal blocks in
    [(h,m',ml), b*64+g] layout (SBUF dst APs cannot split the partition
    dim, DRAM is flat), then — after an explicit barrier, since
    cross-queue DMA->DMA ordering on the staging tile proved racy — 2
    contiguous load-backs into S1BIG[h] [(m'*8+ml) part, b*64+g free];
    batch columns come out g-major (host unpermutes).
  stage 2: 2 matmuls: lhsT = constant mask VM2 [128=(m',ml), 16d],
    rhs = S1BIG[h], K=128, N=512 contiguous, accumulating out4[d, j].

Measured: ~80us HW exec (baseline elementwise kernel: 298us); DMA-bound
(16.8 MB/core at the ~300 GB/s effective DMA-engine ceiling = 56us
transfer + ~9us preamble + ~13us endgame tail).
"""

import sys

import ml_dtypes
import numpy as np

sys.path.insert(0, "/opt/trn_rl_repo")

import concourse.bacc as bacc
import concourse.mybir as mybir
import concourse.tile as tile
from concourse import bass_utils

B, C, H, W = 4096, 512, 4, 4
M = H * W                 # 16 spatial positions
NCORES = 8
BL = B // NCORES          # 512 batches per core
NCH = 4                   # c-chunks of 128 partitions
CH = C // NCH             # 128
FW = BL * M               # 8192 free width of an input chunk
SF = FW // 4              # sub-load width (16 packs)
NPK = BL // 8             # 64 packs of 8 batches

_cached_nc = None


def _make_vm2() -> np.ndarray:
    """vm2[h*128 + m'*8 + ml, d]: pattern mask over (m', m=8h+ml) pairs."""
    vm2 = np.zeros((256, 16), np.float32)
    for mp in range(16):
        for m in range(16):
            y, x = m // 4, m % 4
            for i in range(4):
                for j in range(4):
                    yy, xx = y + i - 2, x + j - 2
                    if 0 <= yy < 4 and 0 <= xx < 4 and mp == yy * 4 + xx:
                        h, ml = m // 8, m % 8
                        vm2[h * 128 + mp * 8 + ml, i * 4 + j] = 1.0
    return vm2.astype(ml_dtypes.bfloat16)


def _build():
    nc = bacc.Bacc("TRN2", target_bir_lowering=False, debug=False)
    f1d = nc.dram_tensor("f1t", [C, FW], mybir.dt.bfloat16, kind="ExternalInput").ap()
    f2d = nc.dram_tensor("f2t", [C, FW], mybir.dt.bfloat16, kind="ExternalInput").ap()
    vmd = nc.dram_tensor("vm2", [256, 16], mybir.dt.bfloat16, kind="ExternalInput").ap()
    outd = nc.dram_tensor("out", [16, BL], mybir.dt.float32, kind="ExternalOutput").ap()

    with tile.TileContext(nc) as tc:
        with (
            tc.tile_pool(name="inp", bufs=1) as inp,
            tc.tile_pool(name="s1g", bufs=1) as s1gp,
            tc.tile_pool(name="sb", bufs=1) as sbp,
            tc.tile_pool(name="vmp", bufs=1) as vmp,
            tc.tile_pool(name="stg", bufs=1, space="DRAM") as stgp,
            tc.tile_pool(name="ps", bufs=6, space="PSUM") as psp,
            tc.tile_pool(name="po", bufs=1, space="PSUM") as pop,
        ):
            t1 = [[None] * 4 for _ in range(NCH)]
            t2 = [[None] * 4 for _ in range(NCH)]
            for q in range(NCH):
                for s in range(4):
                    for tlist, name in ((t1, "t1"), (t2, "t2")):
                        tlist[q][s] = inp.tile(
                            [CH, SF], mybir.dt.bfloat16,
                            tag=f"{name}_{q}_{s}", name=f"{name}_{q}_{s}")

            # blocked Gram accumulator, scatter layout:
            # [16b+m' partitions, m*512 + b*64 + g free]
            s1g = s1gp.tile([128, FW], mybir.dt.bfloat16, tag="s1g",
                            name="s1g")
            # gathered diagonal blocks: [(m'*8+ml) part, b*64+g free]
            sbt = [sbp.tile([128, BL], mybir.dt.bfloat16, tag=f"sb{h}",
                            name=f"sb{h}") for h in range(2)]
            # DRAM staging, flat rows (h, m', ml)
            stg = stgp.tile([256, BL], mybir.dt.bfloat16, tag="stg",
                            name="stg")

            def emit_loads(q, s):
                nc.sync.dma_start(
                    out=t1[q][s][:],
                    in_=f1d[q * CH:(q + 1) * CH, s * SF:(s + 1) * SF])
                nc.gpsimd.dma_start(
                    out=t2[q][s][:],
                    in_=f2d[q * CH:(q + 1) * CH, s * SF:(s + 1) * SF])

            vm2t = [vmp.tile([128, 16], mybir.dt.bfloat16, tag=f"vm{h}",
                             name=f"vm{h}") for h in range(2)]

            for q in range(NCH):
                for s in range(4):
                    emit_loads(q, s)
                    if q == 0 and s == 0:
                        for h in range(2):
                            nc.scalar.dma_start(
                                out=vm2t[h][:],
                                in_=vmd[h * 128:(h + 1) * 128, :])
                    for bk in range(4 * s, 4 * s + 4):
                        pt = psp.tile([128, 512], mybir.dt.float32, tag="pt",
                                      name=f"pt_{q}_{bk}")
                        for j in range(4):
                            go = (bk * 4 + j) % 16
                            sl = slice(go * 128, (go + 1) * 128)
                            nc.tensor.matmul(
                                pt[:, j * 128:(j + 1) * 128],
                                t1[q][s][:, sl], t2[q][s][:, sl],
                                start=True, stop=True,
                            )
                        # scatter-evacuate: dst free dims (m, b, pack)
                        src = pt.rearrange("p (j b m) -> p m b j", j=4, b=8,
                                           m=M)
                        dst = s1g.rearrange(
                            "p (m b g) -> p m b g", m=M, b=8,
                            g=NPK)[:, :, :, 4 * bk:4 * bk + 4]
                        if q == 0:
                            nc.scalar.activation(
                                out=dst, in_=src,
                                func=mybir.ActivationFunctionType.Copy)
                        else:
                            nc.vector.tensor_tensor(
                                out=dst, in0=src, in1=dst,
                                op=mybir.AluOpType.add)

            # gather: stg[(h,m',ml), b*64+g] = s1g[16b+m', (8h+ml)*512+b*64+g]
            # (one DMA per (h, b), split over the sync and gpsimd rings)
            sgv = s1g.rearrange("p (hh ml b2 g) -> p hh ml b2 g",
                                hh=2, ml=8, b2=8, g=NPK)
            dv = stg.rearrange("(hh mp ml) (b2 g) -> hh mp ml b2 g",
                               hh=2, mp=M, ml=8, b2=8)
            engs = [nc.sync, nc.gpsimd, nc.scalar]
            ng = 0
            for h in range(2):
                for b in range(8):
                    engs[ng % 3].dma_start(
                        out=dv[h, :, :, b, :],
                        in_=sgv[16 * b:16 * (b + 1), h, :, b, :])
                    ng += 1
            # cross-queue DMA->DMA ordering on the DRAM staging tile is not
            # reliably enforced by the auto-inserted waits; fence explicitly
            tc.strict_bb_all_engine_barrier()
            for h in range(2):
                (nc.sync if h == 0 else nc.gpsimd).dma_start(
                    out=sbt[h][:], in_=stg[128 * h:128 * (h + 1), :])

            # stage 2: 2 accumulating matmuls, K=128, N=512 contiguous
            out4 = pop.tile([16, BL], mybir.dt.float32, tag="out4",
                            name="out4")
            for h in range(2):
                nc.tensor.matmul(
                    out4[:], vm2t[h][:], sbt[h][:],
                    start=(h == 0), stop=(h == 1),
                )

            ot = vmp.tile([16, BL], mybir.dt.float32, tag="ot", name="ot")
            nc.scalar.activation(out=ot[:], in_=out4[:],
                                 func=mybir.ActivationFunctionType.Copy)
            nc.scalar.dma_start(out=outd, in_=ot[:])

    nc.compile()
    return nc


def _get_nc():
    global _cached_nc
    if _cached_nc is None:
        _cached_nc = _build()
    return _cached_nc


def _prep_core(f: np.ndarray, k: int) -> np.ndarray:
    """Core k's shard as [C, BL*16] bf16 (c-major)."""
    s = f[k * BL:(k + 1) * BL]                      # [BL, C, 16]
    return np.ascontiguousarray(
        s.transpose(1, 0, 2).reshape(C, FW)).astype(ml_dtypes.bfloat16)


# batch index of output column j = b*64+g  ->  batch 8*g + b
_JPERM = 8 * (np.arange(BL) % 64) + np.arange(BL) // 64


def kernel(feat1, feat2):
    f1 = np.asarray(feat1, dtype=np.float32).reshape(B, C, M)
    f2 = np.asarray(feat2, dtype=np.float32).reshape(B, C, M)
    vm2 = _make_vm2()
    nc = _get_nc()
    in_maps = [
        {"f1t": _prep_core(f1, k), "f2t": _prep_core(f2, k), "vm2": vm2}
        for k in range(NCORES)
    ]
    res = bass_utils.run_bass_kernel_spmd(nc, in_maps, list(range(NCORES)))
    out = np.empty((B, M), np.float32)
    for k in range(NCORES):
        r = np.asarray(res.results[k]["out"]).T    # [512 j, 16 d]
        out[k * BL + _JPERM] = r
    return np.ascontiguousarray(out).reshape(B, 1, H, W)
